# revision 9
# baseline (speedup 1.0000x reference)
"""Causal self-attention (B=2, T=2048, C=1024, 16 heads x 64) on 8 TRN2 cores.

Sharding: tensor-parallel over heads (2 heads/core). Each core computes its
heads' QKV projection, causal attention, and a partial output projection
(contraction over its 128 attn columns); the host sums the 8 partials
(row-parallel all-reduce at gather time).

Per-core kernel layout (v2, PE-warmth/weight-reuse optimized):
  - x pre-transposed on host to xT [ci=128, co=8, B*T] (c = co*128+ci).
  - qT/kT/vT [f, t] computed with c-outer loops (stationary weight reused
    across 4 moving chunks); V additionally PE-transposed to t-major with
    an appended ones column.
  - Scores computed transposed, ST[k, q] = KT^T @ QT; the two heads' K=64
    matmuls are emitted alternately so they row-pack into the 128x128 PE
    array concurrently.
  - exp via one ACT pass per [128, 2, 512] group, PSUM->SBUF bf16.
  - No max-subtraction (scores ~N(0,1); exp safe in fp32).
  - PV accumulates [65, q]: V ones-column makes row 64 the softmax
    denominator l[q]. PV is ragged on the causal diagonal band.
  - Normalization (1/l) via reciprocal_approx_fast + K=2-style broadcast
    matmul (sel65) + one DVE multiply into attnT.
  - Output projection per q-chunk right after normalization (keeps PE
    busy through phase transitions); emits out[t, co] fp32 partials.
"""

import os

import numpy as np
import ml_dtypes

B = 2
T = 2048
C = 1024
N_HEADS = 16
D = 64
NCORES = 8
P = 128
BT = B * T
SCALE = D ** -0.5

_bf16 = ml_dtypes.bfloat16

_COMPILED = None
LAST_RESULTS = None  # stashed BassKernelResults for test harness introspection


def _build():
    import concourse.bass as bass
    import concourse.mybir as mybir
    import concourse.tile as tile
    from concourse import bacc

    f32 = mybir.dt.float32
    bf16 = mybir.dt.bfloat16

    nc = bacc.Bacc("TRN2", target_bir_lowering=False, debug=False,
                   num_devices=NCORES)

    xT_d = nc.dram_tensor("xT", [P, 8, BT], bf16, kind="ExternalInput")
    wqkvT_d = nc.dram_tensor("wqkvT", [P, 8, 384], bf16, kind="ExternalInput")
    woutT_d = nc.dram_tensor("woutT", [P, C], bf16, kind="ExternalInput")
    maskT_d = nc.dram_tensor("maskT", [P, P], bf16, kind="ExternalInput")
    sel2_d = nc.dram_tensor("sel2", [65, P], f32, kind="ExternalInput")
    ident_d = nc.dram_tensor("ident", [P, P], bf16, kind="ExternalInput")
    out_d = nc.dram_tensor("out", [BT, C], f32, kind="ExternalOutput")

    Exp = mybir.ActivationFunctionType.Exp

    with tile.TileContext(nc) as tc:
        with (
            tc.tile_pool(name="const", bufs=1) as const_pool,
            tc.tile_pool(name="xT", bufs=2) as xT_pool,
            tc.tile_pool(name="qkv", bufs=2) as qkv_pool,
            tc.tile_pool(name="pt", bufs=4) as pt_pool,
            tc.tile_pool(name="attnT", bufs=2) as attnT_pool,
            tc.tile_pool(name="rl", bufs=2) as rl_pool,
            tc.tile_pool(name="osb", bufs=3) as osb_pool,
            tc.tile_pool(name="st", bufs=2, space="PSUM") as st_pool,
            tc.tile_pool(name="ps4", bufs=4, space="PSUM") as ps4_pool,
        ):
            wqkvT = const_pool.tile([P, 8, 384], bf16, tag="wqkvT")
            woutT = const_pool.tile([P, C], bf16, tag="woutT")
            maskT = const_pool.tile([P, P], bf16, tag="maskT")
            sel2 = const_pool.tile([65, P], f32, tag="sel2")
            ident = const_pool.tile([P, P], bf16, tag="ident")
            nc.sync.dma_start(wqkvT, wqkvT_d[:])
            nc.sync.dma_start(woutT, woutT_d[:])
            nc.sync.dma_start(maskT, maskT_d[:])
            nc.sync.dma_start(sel2, sel2_d[:])
            nc.sync.dma_start(ident, ident_d[:])

            for b in range(B):
                xb = xT_pool.tile([P, 8, T], bf16, tag="xT")
                nc.sync.dma_start(xb, xT_d[:, :, b * T:(b + 1) * T])

                # ---- QKV projection: c-outer so the stationary weight is
                # reused across the 4 moving chunks of each projection.
                qT = qkv_pool.tile([P, T], bf16, tag="qT")
                kT = qkv_pool.tile([P, T], bf16, tag="kT")
                vT = qkv_pool.tile([P, T], bf16, tag="vT")
                for fi, dest in ((0, qT), (1, kT), (2, vT)):
                    pss = [ps4_pool.tile([P, 512], f32, tag="ps4",
                                         name=f"qkvps{n}")
                           for n in range(4)]
                    for c in range(8):
                        for n in range(4):
                            nc.tensor.matmul(
                                pss[n],
                                wqkvT[:, c, fi * 128:(fi + 1) * 128],
                                xb[:, c, n * 512:(n + 1) * 512],
                                start=(c == 0), stop=(c == 7),
                            )
                    for n in range(4):
                        nc.scalar.copy(dest[:, n * 512:(n + 1) * 512], pss[n])

                # V to t-major (PE transpose) with ones column appended.
                vh = [qkv_pool.tile([P, 16, 65], bf16, tag=f"v{h}",
                                    name=f"vh{h}")
                      for h in range(2)]
                for h in range(2):
                    nc.vector.memset(vh[h][:, :, 64], 1.0)
                for tch in range(16):
                    tp = ps4_pool.tile([P, P], bf16, tag="ps4", name="vtp")
                    nc.tensor.transpose(
                        tp, vT[:, tch * 128:(tch + 1) * 128], ident)
                    nc.scalar.copy(vh[0][:, tch, 0:64], tp[:, 0:64])
                    nc.scalar.copy(vh[1][:, tch, 0:64], tp[:, 64:128])

                # ---- attention (heads interleaved for PE row-packing) ----
                attnT = attnT_pool.tile([P, T], bf16, tag="attnT")
                rl2 = rl_pool.tile([65, T], f32, tag="rl2")
                l2 = rl_pool.tile([65, T], f32, tag="l2")
                # rows 1-63 feed zero sel2 rows; 1.0 keeps 1/x finite there
                nc.vector.memset(l2, 1.0)

                for qc in range(4):
                    nk = 4 * qc + 4
                    qsl = slice(qc * 512, (qc + 1) * 512)
                    pv = [ps4_pool.tile([P, 512], f32, tag="ps4",
                                        name=f"pv{h}")
                          for h in range(2)]
                    for g0 in range(0, nk, 2):
                        kbs = list(range(g0, min(g0 + 2, nk)))
                        ng = len(kbs)
                        st = [st_pool.tile([P, 2, 512], f32, tag="st",
                                           name=f"st{h}")
                              for h in range(2)]
                        pt = [pt_pool.tile([P, 2, 512], bf16, tag="pt",
                                           name=f"pt{h}")
                              for h in range(2)]
                        # alternate heads so K=64 matmuls pack in the array
                        for j, kb in enumerate(kbs):
                            for h in range(2):
                                hs = h * 64
                                nc.tensor.matmul(
                                    st[h][:, j, :],
                                    kT[hs:hs + 64, kb * 128:(kb + 1) * 128],
                                    qT[hs:hs + 64, qsl],
                                    start=True, stop=True,
                                )
                        for h in range(2):
                            nc.scalar.activation(
                                pt[h][:, :ng, :], st[h][:, :ng, :], Exp,
                                scale=SCALE)
                        for j, kb in enumerate(kbs):
                            if kb >= 4 * qc:
                                off = (kb - 4 * qc) * 128
                                for h in range(2):
                                    nc.vector.tensor_mul(
                                        pt[h][:, j, off:off + 128],
                                        pt[h][:, j, off:off + 128],
                                        maskT,
                                    )
                        for j, kb in enumerate(kbs):
                            off = max(0, (kb - 4 * qc) * 128)
                            for h in range(2):
                                nc.tensor.matmul(
                                    pv[h][:65, off:512],
                                    vh[h][:, kb, :],
                                    pt[h][:, j, off:512],
                                    start=(kb == 0), stop=(kb == nk - 1),
                                    skip_group_check=True,
                                )
                    # drain: denominators + unnormalized attnT.
                    # NOTE: custom-DVE ops (reciprocal_approx_*) mishandle
                    # non-zero partition bases on HW — move l to a base-0
                    # SBUF tile with regular copies first.
                    for h in range(2):
                        hs = h * 64
                        nc.vector.tensor_copy(
                            l2[hs:hs + 1, qsl], pv[h][64:65, :])
                        nc.vector.tensor_copy(
                            attnT[hs:hs + 64, qsl], pv[h][0:64, :])
                    nc.vector.reciprocal_approx_fast(
                        rl2[:, qsl], l2[:, qsl])
                    # normalize this q-chunk
                    rb = ps4_pool.tile([P, 512], f32, tag="ps4", name="rb")
                    nc.tensor.matmul(rb, sel2[:, :], rl2[:, qsl],
                                     start=True, stop=True)
                    nc.vector.tensor_mul(attnT[:, qsl], attnT[:, qsl], rb)

                    # output projection for this q-chunk's 4 token blocks
                    for tb in range(4 * qc, 4 * qc + 4):
                        ps_a = ps4_pool.tile([P, 512], f32, tag="ps4",
                                             name="opa")
                        ps_b = ps4_pool.tile([P, 512], f32, tag="ps4",
                                             name="opb")
                        nc.tensor.matmul(
                            ps_a, attnT[:, tb * 128:(tb + 1) * 128],
                            woutT[:, 0:512], start=True, stop=True)
                        nc.tensor.matmul(
                            ps_b, attnT[:, tb * 128:(tb + 1) * 128],
                            woutT[:, 512:1024], start=True, stop=True)
                        osb = osb_pool.tile([P, C], f32, tag="osb")
                        nc.vector.tensor_copy(osb[:, 0:512], ps_a)
                        nc.vector.tensor_copy(osb[:, 512:1024], ps_b)
                        nc.sync.dma_start(
                            out_d[(b * T + tb * 128):
                                  (b * T + (tb + 1) * 128), :],
                            osb)

    nc.compile()
    return nc


def _get_compiled():
    global _COMPILED
    if _COMPILED is None:
        _COMPILED = _build()
    return _COMPILED


def make_core_inputs(x, w_qkv, w_out):
    """Host-side shard prep: returns list of per-core input dicts."""
    xf = np.asarray(x, dtype=np.float32).reshape(BT, C)
    xT = np.ascontiguousarray(
        xf.T.reshape(8, P, BT).transpose(1, 0, 2)).astype(_bf16)

    maskT = np.zeros((P, P), dtype=_bf16)
    kk, qq = np.meshgrid(np.arange(P), np.arange(P), indexing="ij")
    maskT[kk <= qq] = 1.0

    sel2 = np.zeros((65, P), dtype=np.float32)
    sel2[0, 0:64] = 1.0
    sel2[64, 64:128] = 1.0

    ident = np.eye(P, dtype=_bf16)

    w_qkv = np.asarray(w_qkv, dtype=np.float32)
    w_out = np.asarray(w_out, dtype=np.float32)

    ins = []
    for core in range(NCORES):
        r0 = 2 * core * D
        wsel = np.concatenate([
            w_qkv[r0:r0 + 128],
            w_qkv[C + r0:C + r0 + 128],
            w_qkv[2 * C + r0:2 * C + r0 + 128],
        ], axis=0)  # [384, 1024]
        wqkvT = np.ascontiguousarray(
            wsel.T.reshape(8, P, 384).transpose(1, 0, 2)).astype(_bf16)
        woutT = np.ascontiguousarray(
            w_out[:, core * P:(core + 1) * P].T).astype(_bf16)
        ins.append({
            "xT": xT,
            "wqkvT": wqkvT,
            "woutT": woutT,
            "maskT": maskT,
            "sel2": sel2,
            "ident": ident,
        })
    return ins


def kernel(x, w_qkv, w_out):
    global LAST_RESULTS
    from concourse.bass_utils import run_bass_kernel_spmd

    nc = _get_compiled()
    ins = make_core_inputs(x, w_qkv, w_out)
    trace = bool(os.environ.get("KERNEL_TRACE"))
    res = run_bass_kernel_spmd(nc, ins, core_ids=list(range(NCORES)),
                               trace=trace)
    LAST_RESULTS = res
    out = np.zeros((BT, C), dtype=np.float32)
    for r in res.results:
        out += r["out"]
    return out.reshape(B, T, C)


# revision 10
# speedup vs baseline: 1.0411x; 1.0411x over previous
"""Causal self-attention (B=2, T=2048, C=1024, 16 heads x 64) on 8 TRN2 cores.

Sharding: tensor-parallel over heads (2 heads/core). Each core computes its
heads' QKV projection, causal attention, and a partial output projection
(contraction over its 128 attn columns); the host sums the 8 partials
(row-parallel all-reduce at gather time).

Per-core kernel layout (v2, PE-warmth/weight-reuse optimized):
  - x pre-transposed on host to xT [ci=128, co=8, B*T] (c = co*128+ci).
  - qT/kT/vT [f, t] computed with c-outer loops (stationary weight reused
    across 4 moving chunks); V additionally PE-transposed to t-major with
    an appended ones column.
  - Scores computed transposed, ST[k, q] = KT^T @ QT; the two heads' K=64
    matmuls are emitted alternately so they row-pack into the 128x128 PE
    array concurrently.
  - exp via one ACT pass per [128, 2, 512] group, PSUM->SBUF bf16.
  - No max-subtraction (scores ~N(0,1); exp safe in fp32).
  - PV accumulates [65, q]: V ones-column makes row 64 the softmax
    denominator l[q]. PV is ragged on the causal diagonal band.
  - Normalization (1/l) via reciprocal_approx_fast + K=2-style broadcast
    matmul (sel65) + one DVE multiply into attnT.
  - Output projection per q-chunk right after normalization (keeps PE
    busy through phase transitions); emits out[t, co] fp32 partials.
"""

import os

import numpy as np
import ml_dtypes

B = 2
T = 2048
C = 1024
N_HEADS = 16
D = 64
NCORES = 8
P = 128
BT = B * T
SCALE = D ** -0.5

_bf16 = ml_dtypes.bfloat16

_COMPILED = None
LAST_RESULTS = None  # stashed BassKernelResults for test harness introspection


def _build():
    import concourse.bass as bass
    import concourse.mybir as mybir
    import concourse.tile as tile
    from concourse import bacc

    f32 = mybir.dt.float32
    bf16 = mybir.dt.bfloat16

    nc = bacc.Bacc("TRN2", target_bir_lowering=False, debug=False,
                   num_devices=NCORES)

    xT_d = nc.dram_tensor("xT", [P, 8, BT], bf16, kind="ExternalInput")
    wqkvT_d = nc.dram_tensor("wqkvT", [P, 8, 384], bf16, kind="ExternalInput")
    woutT_d = nc.dram_tensor("woutT", [P, C], bf16, kind="ExternalInput")
    maskT_d = nc.dram_tensor("maskT", [P, P], bf16, kind="ExternalInput")
    sel2_d = nc.dram_tensor("sel2", [65, P], f32, kind="ExternalInput")
    ident_d = nc.dram_tensor("ident", [P, P], bf16, kind="ExternalInput")
    out_d = nc.dram_tensor("out", [BT, C], f32, kind="ExternalOutput")

    Exp = mybir.ActivationFunctionType.Exp

    with tile.TileContext(nc) as tc:
        with (
            tc.tile_pool(name="const", bufs=1) as const_pool,
            tc.tile_pool(name="xT", bufs=2) as xT_pool,
            tc.tile_pool(name="qkv", bufs=2) as qkv_pool,
            tc.tile_pool(name="pt", bufs=4) as pt_pool,
            tc.tile_pool(name="attnT", bufs=2) as attnT_pool,
            tc.tile_pool(name="rl", bufs=2) as rl_pool,
            tc.tile_pool(name="osb", bufs=3) as osb_pool,
            tc.tile_pool(name="st", bufs=2, space="PSUM") as st_pool,
            tc.tile_pool(name="ps4", bufs=4, space="PSUM") as ps4_pool,
        ):
            wqkvT = const_pool.tile([P, 8, 384], bf16, tag="wqkvT")
            woutT = const_pool.tile([P, C], bf16, tag="woutT")
            maskT = const_pool.tile([P, P], bf16, tag="maskT")
            sel2 = const_pool.tile([65, P], f32, tag="sel2")
            ident = const_pool.tile([P, P], bf16, tag="ident")
            nc.sync.dma_start(wqkvT, wqkvT_d[:])
            nc.sync.dma_start(woutT, woutT_d[:])
            nc.sync.dma_start(maskT, maskT_d[:])
            nc.sync.dma_start(sel2, sel2_d[:])
            nc.sync.dma_start(ident, ident_d[:])

            for b in range(B):
                xb = xT_pool.tile([P, 8, T], bf16, tag="xT")
                nc.sync.dma_start(xb, xT_d[:, :, b * T:(b + 1) * T])

                # ---- QKV projection: c-outer so the stationary weight is
                # reused across the 4 moving chunks of each projection.
                qT = qkv_pool.tile([P, T], bf16, tag="qT")
                kT = qkv_pool.tile([P, T], bf16, tag="kT")
                vT = qkv_pool.tile([P, T], bf16, tag="vT")
                for fi, dest in ((0, qT), (1, kT), (2, vT)):
                    pss = [ps4_pool.tile([P, 512], f32, tag="ps4",
                                         name=f"qkvps{n}")
                           for n in range(4)]
                    for c in range(8):
                        for n in range(4):
                            nc.tensor.matmul(
                                pss[n],
                                wqkvT[:, c, fi * 128:(fi + 1) * 128],
                                xb[:, c, n * 512:(n + 1) * 512],
                                start=(c == 0), stop=(c == 7),
                            )
                    for n in range(4):
                        nc.scalar.copy(dest[:, n * 512:(n + 1) * 512], pss[n])

                # V to t-major (PE transpose) with ones column appended.
                vh = [qkv_pool.tile([P, 16, 65], bf16, tag=f"v{h}",
                                    name=f"vh{h}")
                      for h in range(2)]
                for h in range(2):
                    nc.vector.memset(vh[h][:, :, 64], 1.0)
                for tch in range(16):
                    tp = ps4_pool.tile([P, P], bf16, tag="ps4", name="vtp")
                    nc.tensor.transpose(
                        tp, vT[:, tch * 128:(tch + 1) * 128], ident)
                    nc.scalar.copy(vh[0][:, tch, 0:64], tp[:, 0:64])
                    nc.scalar.copy(vh[1][:, tch, 0:64], tp[:, 64:128])

                # ---- attention (heads interleaved for PE row-packing) ----
                attnT = attnT_pool.tile([P, T], bf16, tag="attnT")
                rl2 = rl_pool.tile([65, T], f32, tag="rl2")
                l2 = rl_pool.tile([65, T], f32, tag="l2")
                # rows 1-63 feed zero sel2 rows; 1.0 keeps 1/x finite there
                nc.vector.memset(l2, 1.0)

                def finish_qc(qc):
                    # deferred by one q-chunk: deps are long satisfied, so
                    # these PE ops never stall (keeps HAM warm)
                    qsl = slice(qc * 512, (qc + 1) * 512)
                    nc.vector.reciprocal_approx_fast(
                        rl2[:, qsl], l2[:, qsl])
                    rb = ps4_pool.tile([P, 512], f32, tag="ps4", name="rb")
                    nc.tensor.matmul(rb, sel2[:, :], rl2[:, qsl],
                                     start=True, stop=True)
                    nc.vector.tensor_mul(attnT[:, qsl], attnT[:, qsl], rb)
                    for tb in range(4 * qc, 4 * qc + 4):
                        ps_a = ps4_pool.tile([P, 512], f32, tag="ps4",
                                             name="opa")
                        ps_b = ps4_pool.tile([P, 512], f32, tag="ps4",
                                             name="opb")
                        nc.tensor.matmul(
                            ps_a, attnT[:, tb * 128:(tb + 1) * 128],
                            woutT[:, 0:512], start=True, stop=True)
                        nc.tensor.matmul(
                            ps_b, attnT[:, tb * 128:(tb + 1) * 128],
                            woutT[:, 512:1024], start=True, stop=True)
                        osb = osb_pool.tile([P, C], f32, tag="osb")
                        nc.vector.tensor_copy(osb[:, 0:512], ps_a)
                        nc.vector.tensor_copy(osb[:, 512:1024], ps_b)
                        nc.sync.dma_start(
                            out_d[(b * T + tb * 128):
                                  (b * T + (tb + 1) * 128), :],
                            osb)

                for qc in range(4):
                    nk = 4 * qc + 4
                    qsl = slice(qc * 512, (qc + 1) * 512)
                    pv = [ps4_pool.tile([P, 512], f32, tag="ps4",
                                        name=f"pv{h}")
                          for h in range(2)]
                    for g0 in range(0, nk, 2):
                        kbs = list(range(g0, min(g0 + 2, nk)))
                        ng = len(kbs)
                        st = [st_pool.tile([P, 2, 512], f32, tag="st",
                                           name=f"st{h}")
                              for h in range(2)]
                        pt = [pt_pool.tile([P, 2, 512], bf16, tag="pt",
                                           name=f"pt{h}")
                              for h in range(2)]
                        # alternate heads so K=64 matmuls pack in the array
                        for j, kb in enumerate(kbs):
                            for h in range(2):
                                hs = h * 64
                                nc.tensor.matmul(
                                    st[h][:, j, :],
                                    kT[hs:hs + 64, kb * 128:(kb + 1) * 128],
                                    qT[hs:hs + 64, qsl],
                                    start=True, stop=True,
                                )
                        for h in range(2):
                            nc.scalar.activation(
                                pt[h][:, :ng, :], st[h][:, :ng, :], Exp,
                                scale=SCALE)
                        for j, kb in enumerate(kbs):
                            if kb >= 4 * qc:
                                off = (kb - 4 * qc) * 128
                                for h in range(2):
                                    nc.vector.tensor_mul(
                                        pt[h][:, j, off:off + 128],
                                        pt[h][:, j, off:off + 128],
                                        maskT,
                                    )
                        for j, kb in enumerate(kbs):
                            off = max(0, (kb - 4 * qc) * 128)
                            for h in range(2):
                                nc.tensor.matmul(
                                    pv[h][:65, off:512],
                                    vh[h][:, kb, :],
                                    pt[h][:, j, off:512],
                                    start=(kb == 0), stop=(kb == nk - 1),
                                    skip_group_check=True,
                                )
                    # drain: denominators + unnormalized attnT.
                    # NOTE: custom-DVE ops (reciprocal_approx_*) mishandle
                    # non-zero partition bases on HW — move l to a base-0
                    # SBUF tile with regular copies first.
                    for h in range(2):
                        hs = h * 64
                        nc.vector.tensor_copy(
                            l2[hs:hs + 1, qsl], pv[h][64:65, :])
                        nc.vector.tensor_copy(
                            attnT[hs:hs + 64, qsl], pv[h][0:64, :])
                    if qc > 0:
                        finish_qc(qc - 1)
                if True:
                    finish_qc(3)

    nc.compile()
    return nc


def _get_compiled():
    global _COMPILED
    if _COMPILED is None:
        _COMPILED = _build()
    return _COMPILED


def make_core_inputs(x, w_qkv, w_out):
    """Host-side shard prep: returns list of per-core input dicts."""
    xf = np.asarray(x, dtype=np.float32).reshape(BT, C)
    xT = np.ascontiguousarray(
        xf.T.reshape(8, P, BT).transpose(1, 0, 2)).astype(_bf16)

    maskT = np.zeros((P, P), dtype=_bf16)
    kk, qq = np.meshgrid(np.arange(P), np.arange(P), indexing="ij")
    maskT[kk <= qq] = 1.0

    sel2 = np.zeros((65, P), dtype=np.float32)
    sel2[0, 0:64] = 1.0
    sel2[64, 64:128] = 1.0

    ident = np.eye(P, dtype=_bf16)

    w_qkv = np.asarray(w_qkv, dtype=np.float32)
    w_out = np.asarray(w_out, dtype=np.float32)

    ins = []
    for core in range(NCORES):
        r0 = 2 * core * D
        wsel = np.concatenate([
            w_qkv[r0:r0 + 128],
            w_qkv[C + r0:C + r0 + 128],
            w_qkv[2 * C + r0:2 * C + r0 + 128],
        ], axis=0)  # [384, 1024]
        wqkvT = np.ascontiguousarray(
            wsel.T.reshape(8, P, 384).transpose(1, 0, 2)).astype(_bf16)
        woutT = np.ascontiguousarray(
            w_out[:, core * P:(core + 1) * P].T).astype(_bf16)
        ins.append({
            "xT": xT,
            "wqkvT": wqkvT,
            "woutT": woutT,
            "maskT": maskT,
            "sel2": sel2,
            "ident": ident,
        })
    return ins


def kernel(x, w_qkv, w_out):
    global LAST_RESULTS
    from concourse.bass_utils import run_bass_kernel_spmd

    nc = _get_compiled()
    ins = make_core_inputs(x, w_qkv, w_out)
    trace = bool(os.environ.get("KERNEL_TRACE"))
    res = run_bass_kernel_spmd(nc, ins, core_ids=list(range(NCORES)),
                               trace=trace)
    LAST_RESULTS = res
    out = np.zeros((BT, C), dtype=np.float32)
    for r in res.results:
        out += r["out"]
    return out.reshape(B, T, C)
